# revision 10
# baseline (speedup 1.0000x reference)
"""Trainium2 Bass kernel for a CustomSAGEConv (two LSTM-reduced GNN stages).

Computation (N nodes, D neighbors each, F features, H = F):
    mail1   = x[src]                          # [N, D, F]
    h_neigh = LSTM1(mail1).h_final            # [N, F]
    h       = x @ W_self + h_neigh @ W_neigh + b
    mail2   = h[src]
    out     = LSTM2(mail2).h_final            # [N, F]

Strategy (8 NeuronCores, dst-node sharding, weights replicated):
  * The LSTM input projection of a gathered mailbox equals a gather of the
    projected node table: (x[src]) @ Wih.T == (x @ Wih.T)[src].  Each core
    computes the projected table once (21 GFLOP instead of 671 GFLOP)
    and row-gathers [128, 4F] tiles per step with indirect DMA.
  * Recurrent h @ Whh.T runs per-core on the PE in bf16; hidden state is
    re-transposed each step with DMA transposes so it can be the stationary
    operand of the next step.
  * Gates are reordered host-side to [i, f, o, g] so one sigmoid covers
    [0:3F] and one tanh covers [3F:4F] on the scalar (ACT) engine.
  * Cell state c stays fp32; gates accumulate in fp32 PSUM; two of the
    cell muls run on GpSimd to unload the vector engine.
  * Stage-2 projection is node-sharded and AllGathered as a table.

kernel(**inputs) takes full-size numpy inputs and returns the full [N, F]
float32 output.
"""

import math
from contextlib import ExitStack
from dataclasses import dataclass

import ml_dtypes
import numpy as np

import concourse.bass as bass
import concourse.mybir as mybir
import concourse.tile as tile
from concourse import bacc
from concourse.bass_utils import run_bass_kernel_spmd
from concourse.masks import make_identity

BF16 = mybir.dt.bfloat16
F32 = mybir.dt.float32
I32 = mybir.dt.int32
AF = mybir.ActivationFunctionType
NP_BF16 = ml_dtypes.bfloat16


@dataclass(frozen=True)
class Cfg:
    n: int = 10000      # nodes
    d: int = 32         # in-degree (LSTM steps)
    f: int = 512        # features == hidden
    cores: int = 8
    has_b1: bool = False  # nonzero bih1+bhh1
    has_b2: bool = False  # nonzero bih2+bhh2

    @property
    def g(self):  # gate width
        return 4 * self.f

    @property
    def kt(self):  # contraction tiles of 128 over f
        return self.f // 128

    @property
    def npc(self):  # nodes per core
        return self.n // self.cores

    @property
    def mt(self):  # node tiles of 128 per core
        return math.ceil(self.npc / 128)

    @property
    def npad(self):  # padded nodes per core
        return self.mt * 128

    @property
    def mt_all(self):  # node tiles over all nodes (stage-1 table)
        return math.ceil(self.n / 128)

    @property
    def npad_all(self):
        return self.mt_all * 128


def _load_wtile(nc, pool, dram, kt, width, dtype=BF16, name=None):
    """DRAM [kt*128, width] -> SBUF [128, kt*width]; slice k at [:, k*width:+width]."""
    t = pool.tile([128, kt * width], dtype, tag=name, name=name or "wt")
    for k in range(kt):
        nc.sync.dma_start(
            out=t[:, k * width:(k + 1) * width],
            in_=dram[k * 128:(k + 1) * 128, :],
        )
    return t


def _proj_phase(nc, tc, cfg, stat_src, wt_sb, table, mtiles, bias_sb):
    """table[m*128+p, :] = sum_k stat(m,k).T @ w[k]  (+ ones.T @ bias).

    stat_src(m, k) -> AP [128, 128]: DRAM (DMA'd to SBUF here) or SBUF
    (used as the stationary operand directly).
    """
    g = cfg.g
    kt = cfg.kt
    with (
        tc.tile_pool(name="proj_stat", bufs=4) as statp,
        tc.tile_pool(name="proj_psum", bufs=2, space="PSUM") as psump,
        tc.tile_pool(name="proj_out", bufs=3) as outp,
    ):
        ones_sb = None
        if bias_sb is not None:
            ones_sb = statp.tile([1, 128], BF16, tag="ones", name="ones")
            nc.gpsimd.memset(ones_sb[:], 1.0)
        for m in range(mtiles):
            ps = psump.tile([128, g], F32, tag="pj", name="pj")
            for k in range(kt):
                src = stat_src(m, k)
                if src.space == bass.MemorySpace.DRAM:
                    st = statp.tile([128, 128], BF16, tag="st", name="st")
                    nc.sync.dma_start(out=st[:], in_=src)
                    src = st[:]
                for ns in range(g // 512):
                    nc.tensor.matmul(
                        ps[:, ns * 512:(ns + 1) * 512],
                        lhsT=src,
                        rhs=wt_sb[:, k * g + ns * 512: k * g + (ns + 1) * 512],
                        start=(k == 0),
                        stop=(k == kt - 1 and bias_sb is None),
                    )
            if bias_sb is not None:
                for ns in range(g // 512):
                    nc.tensor.matmul(
                        ps[:, ns * 512:(ns + 1) * 512],
                        lhsT=ones_sb[:],
                        rhs=bias_sb[:, ns * 512:(ns + 1) * 512],
                        start=False,
                        stop=True,
                    )
            ot = outp.tile([128, g], BF16, tag="pt", name="pt")
            nc.scalar.copy(ot[:], ps[:])
            nc.sync.dma_start(out=table[m * 128:(m + 1) * 128, :], in_=ot[:])


def _lstm_phase(nc, tc, cfg, table, idx_sb, whh_sb, ident, hT_sb, out_dram):
    """Run cfg.d LSTM steps over this core's cfg.mt node tiles.

    Gate layout is [i, f, o, g].  Hidden state is PE-transposed into hT_sb
    [128, kt*npad] (bf16); if out_dram is given the final hidden state is
    instead written there as fp32 [npc, f].
    """
    f, g, kt, mt, d, npad, npc = cfg.f, cfg.g, cfg.kt, cfg.mt, cfg.d, cfg.npad, cfg.npc
    halves = max(1, g // 1024)
    hw = min(g, 1024)
    hT_v = hT_sb[:].rearrange("p (k n) -> p k n", k=kt)

    with (
        tc.tile_pool(name="lstm_state", bufs=1) as cstp,
        tc.tile_pool(name="lstm_gather", bufs=4) as gp,
        tc.tile_pool(name="lstm_gates", bufs=2) as sp,
        tc.tile_pool(name="lstm_act", bufs=3) as ap_,
        tc.tile_pool(name="lstm_dve", bufs=3) as dp,
        tc.tile_pool(name="lstm_psum", bufs=3, space="PSUM") as pp,
        tc.tile_pool(name="lstm_tpsum", bufs=2, space="PSUM") as tpp,
    ):
        c_st = [cstp.tile([128, f], F32, tag=f"c{m}", name=f"c{m}") for m in range(mt)]
        for t in range(d):
            last = t == d - 1
            for m in range(mt):
                gsb = gp.tile([128, g], BF16, tag="g", name="g")
                nc.gpsimd.indirect_dma_start(
                    out=gsb[:],
                    out_offset=None,
                    in_=table[:, :],
                    in_offset=bass.IndirectOffsetOnAxis(
                        ap=idx_sb[:, t * mt + m: t * mt + m + 1], axis=0
                    ),
                )
                if t > 0:
                    gates = sp.tile([128, g], F32, tag="gt", name="gt")
                    # k-outer so one stationary load feeds all g//512 matmuls
                    pss = [pp.tile([128, hw], F32, tag="ps", name=f"ps{h}")
                           for h in range(halves)]
                    for k in range(kt):
                        lhs = hT_sb[:, k * npad + m * 128: k * npad + (m + 1) * 128]
                        for ns in range(g // 512):
                            h, hns = divmod(ns, hw // 512)
                            nc.tensor.matmul(
                                pss[h][:, hns * 512:(hns + 1) * 512],
                                lhsT=lhs,
                                rhs=whh_sb[:, k * g + ns * 512: k * g + (ns + 1) * 512],
                                start=(k == 0),
                                stop=(k == kt - 1),
                            )
                    for h in range(halves):
                        nc.vector.tensor_add(
                            gates[:, h * hw:(h + 1) * hw], pss[h][:],
                            gsb[:, h * hw:(h + 1) * hw]
                        )
                else:
                    gates = gsb
                # gate order [i, f, o, g]: one sigmoid over 3F, one tanh
                sif = ap_.tile([128, 3 * f], BF16, tag="sif", name="sif")
                tg = ap_.tile([128, f], BF16, tag="tg", name="tg")
                nc.scalar.activation(sif[:], gates[:, 0:3 * f], AF.Sigmoid)
                nc.scalar.activation(tg[:], gates[:, 3 * f:4 * f], AF.Tanh)
                si, sf, so = sif[:, 0:f], sif[:, f:2 * f], sif[:, 2 * f:3 * f]
                if t > 0:
                    t1 = dp.tile([128, f], BF16, tag="t1", name="t1")
                    t2 = dp.tile([128, f], F32, tag="t2", name="t2")
                    nc.gpsimd.tensor_mul(t1[:], si, tg[:])
                    nc.vector.tensor_mul(t2[:], sf, c_st[m][:])
                    nc.vector.tensor_add(c_st[m][:], t1[:], t2[:])
                else:
                    nc.vector.tensor_mul(c_st[m][:], si, tg[:])
                tch = ap_.tile([128, f], BF16, tag="tc", name="tc")
                nc.scalar.activation(tch[:], c_st[m][:], AF.Tanh)
                if last and out_dram is not None:
                    hf = dp.tile([128, f], F32, tag="hf", name="hf")
                    nc.vector.tensor_mul(hf[:], so, tch[:])
                    rows = min(128, npc - m * 128)
                    nc.sync.dma_start(
                        out=out_dram[m * 128: m * 128 + rows, :], in_=hf[:rows, :]
                    )
                else:
                    hm = dp.tile([128, f], BF16, tag="hm", name="hm")
                    nc.gpsimd.tensor_mul(hm[:], so, tch[:])
                    pt = tpp.tile([128, f], BF16, tag="tp", name="tp")
                    for k in range(kt):
                        nc.tensor.transpose(
                            pt[:, k * 128:(k + 1) * 128],
                            hm[:, k * 128:(k + 1) * 128], ident[:]
                        )
                    nc.scalar.copy(
                        hT_v[:, :, m * 128:(m + 1) * 128],
                        pt[:].rearrange("p (k n) -> p k n", k=kt),
                    )


def build(cfg: Cfg):
    nc = bacc.Bacc("TRN2", target_bir_lowering=False, debug=False,
                   num_devices=cfg.cores)
    f, g, kt, mt, d, npad, npc, C = (
        cfg.f, cfg.g, cfg.kt, cfg.mt, cfg.d, cfg.npad, cfg.npc, cfg.cores
    )

    # --- I/O -------------------------------------------------------------
    xT = nc.dram_tensor("xT", [f, cfg.npad_all], BF16, kind="ExternalInput")
    xTme = nc.dram_tensor("xTme", [f, npad], BF16, kind="ExternalInput")
    wih1T = nc.dram_tensor("wih1T", [f, g], BF16, kind="ExternalInput")
    whh1T = nc.dram_tensor("whh1T", [f, g], BF16, kind="ExternalInput")
    wih2T = nc.dram_tensor("wih2T", [f, g], BF16, kind="ExternalInput")
    whh2T = nc.dram_tensor("whh2T", [f, g], BF16, kind="ExternalInput")
    wself = nc.dram_tensor("wself", [f, f], BF16, kind="ExternalInput")
    wneigh = nc.dram_tensor("wneigh", [f, f], BF16, kind="ExternalInput")
    blinT = nc.dram_tensor("blinT", [128, kt], F32, kind="ExternalInput")
    idx1 = nc.dram_tensor("idx1", [128, d * mt], I32, kind="ExternalInput")
    idx2 = nc.dram_tensor("idx2", [128, d * mt], I32, kind="ExternalInput")
    bt1 = bt2 = None
    if cfg.has_b1:
        bt1 = nc.dram_tensor("bt1", [1, g], BF16, kind="ExternalInput")
    if cfg.has_b2:
        bt2 = nc.dram_tensor("bt2", [1, g], BF16, kind="ExternalInput")
    out = nc.dram_tensor("out", [npc, f], F32, kind="ExternalOutput")

    # --- internal DRAM ---------------------------------------------------
    table1 = nc.dram_tensor("table1", [cfg.npad_all, g], BF16)
    table2loc = nc.dram_tensor("table2loc", [npad, g], BF16)
    table2 = nc.dram_tensor("table2", [C * npad, g], BF16, addr_space="Shared")

    with tile.TileContext(nc) as tc, ExitStack() as ctx:
        const = ctx.enter_context(tc.tile_pool(name="const", bufs=1))
        ident = const.tile([128, 128], BF16, tag="ident", name="ident")
        make_identity(nc, ident[:])
        idx1_sb = const.tile([128, d * mt], I32, tag="idx1", name="idx1")
        nc.sync.dma_start(out=idx1_sb[:], in_=idx1[:, :])
        idx2_sb = const.tile([128, d * mt], I32, tag="idx2", name="idx2")
        nc.sync.dma_start(out=idx2_sb[:], in_=idx2[:, :])
        blin_sb = const.tile([128, kt], F32, tag="blin", name="blin")
        nc.sync.dma_start(out=blin_sb[:], in_=blinT[:, :])

        # stage-1 hidden-state (transposed) persists into the linear stage
        s1pool = ctx.enter_context(tc.tile_pool(name="s1", bufs=1))
        hT1 = s1pool.tile([128, kt * npad], BF16, tag="hT1", name="hT1")

        # ---- stage 1: table1 = x @ Wih1.T (+b1) -------------------------
        with tc.tile_pool(name="w1", bufs=1) as w1p:
            wih1_sb = _load_wtile(nc, w1p, wih1T, kt, g, name="wih1")
            b1_sb = None
            if bt1 is not None:
                b1_sb = w1p.tile([1, g], BF16, tag="b1", name="b1")
                nc.sync.dma_start(out=b1_sb[:], in_=bt1[:, :])
            _proj_phase(
                nc, tc, cfg,
                lambda m, k: xT[k * 128:(k + 1) * 128, m * 128:(m + 1) * 128],
                wih1_sb, table1, cfg.mt_all, b1_sb,
            )

        # ---- stage 1: LSTM over mailboxes -------------------------------
        with tc.tile_pool(name="whh1", bufs=1) as whh1p:
            whh1_sb = _load_wtile(nc, whh1p, whh1T, kt, g, name="whh1")
            _lstm_phase(nc, tc, cfg, table1, idx1_sb, whh1_sb, ident, hT1, None)

        # ---- stage 1 linear (transposed) + local stage-2 projection -----
        with tc.tile_pool(name="lin", bufs=1) as linp:
            hlin_sb = linp.tile([128, kt * npad], BF16, tag="hlin", name="hlin")
            with (
                tc.tile_pool(name="lin_w", bufs=1) as linwp,
                tc.tile_pool(name="lin_psum", bufs=2, space="PSUM") as linpp,
            ):
                wself_sb = _load_wtile(nc, linwp, wself, kt, f, name="ws")
                wneigh_sb = _load_wtile(nc, linwp, wneigh, kt, f, name="wn")
                xtme_sb = _load_wtile(nc, linwp, xTme, kt, npad, name="xtme")
                nch = math.ceil(npad / 512)
                for hk in range(kt):
                    for nc_i in range(nch):
                        nw = min(512, npad - nc_i * 512)
                        ps = linpp.tile([128, 512], F32, tag="lp", name="lp")
                        for k in range(kt):
                            nc.tensor.matmul(
                                ps[:, :nw],
                                lhsT=wself_sb[:, k * f + hk * 128: k * f + hk * 128 + 128],
                                rhs=xtme_sb[:, k * npad + nc_i * 512: k * npad + nc_i * 512 + nw],
                                start=(k == 0),
                                stop=False,
                            )
                        for k in range(kt):
                            nc.tensor.matmul(
                                ps[:, :nw],
                                lhsT=wneigh_sb[:, k * f + hk * 128: k * f + hk * 128 + 128],
                                rhs=hT1[:, k * npad + nc_i * 512: k * npad + nc_i * 512 + nw],
                                start=False,
                                stop=(k == kt - 1),
                            )
                        nc.scalar.activation(
                            hlin_sb[:, hk * npad + nc_i * 512: hk * npad + nc_i * 512 + nw],
                            ps[:, :nw], AF.Identity,
                            bias=blin_sb[:, hk:hk + 1],
                        )

            # stage-2 projection of the LOCAL node block (stationary from SBUF)
            with tc.tile_pool(name="w2", bufs=1) as w2p:
                wih2_sb = _load_wtile(nc, w2p, wih2T, kt, g, name="wih2")
                b2_sb = None
                if bt2 is not None:
                    b2_sb = w2p.tile([1, g], BF16, tag="b2", name="b2")
                    nc.sync.dma_start(out=b2_sb[:], in_=bt2[:, :])
                _proj_phase(
                    nc, tc, cfg,
                    lambda m, k: hlin_sb[:, k * npad + m * 128:
                                         k * npad + (m + 1) * 128],
                    wih2_sb, table2loc, mt, b2_sb,
                )

        # ---- all-gather the stage-2 table -------------------------------
        nc.gpsimd.collective_compute(
            "AllGather",
            mybir.AluOpType.bypass,
            replica_groups=[list(range(C))],
            ins=[table2loc[:, :]],
            outs=[table2[:, :]],
        )

        # ---- stage 2: LSTM -> out ---------------------------------------
        with (
            tc.tile_pool(name="whh2", bufs=1) as whh2p,
            tc.tile_pool(name="s2", bufs=1) as s2pool,
        ):
            whh2_sb = _load_wtile(nc, whh2p, whh2T, kt, g, name="whh2")
            hT2 = s2pool.tile([128, kt * npad], BF16, tag="hT2", name="hT2")
            _lstm_phase(nc, tc, cfg, table2, idx2_sb, whh2_sb, ident, hT2, out)

    nc.compile()
    return nc


_CACHE: dict = {}


def _perm_gates(w):
    """Reorder gate blocks [i, f, g, o] -> [i, f, o, g] along axis 0."""
    i, f_, g, o = np.split(np.asarray(w), 4, 0)
    return np.concatenate([i, f_, o, g], 0)


def _prep_inputs(cfg: Cfg, x, src, Wih1, Whh1, bih1, bhh1, W_self, W_neigh, b,
                 Wih2, Whh2, bih2, bhh2):
    """Build the 8 per-core input maps (host-side slicing/transposition only)."""
    f, g, d, mt, npc, npad, C = cfg.f, cfg.g, cfg.d, cfg.mt, cfg.npc, cfg.npad, cfg.cores

    def bf(a):
        return np.ascontiguousarray(a, dtype=np.float32).astype(NP_BF16)

    xT = np.zeros((f, cfg.npad_all), NP_BF16)
    xT[:, :cfg.n] = bf(x.T)
    shared = {
        "xT": xT,
        "wih1T": bf(_perm_gates(Wih1).T), "whh1T": bf(_perm_gates(Whh1).T),
        "wih2T": bf(_perm_gates(Wih2).T), "whh2T": bf(_perm_gates(Whh2).T),
        "wself": bf(W_self), "wneigh": bf(W_neigh),
        "blinT": np.ascontiguousarray(
            np.asarray(b, np.float32).reshape(cfg.kt, 128).T),
    }
    if cfg.has_b1:
        shared["bt1"] = bf(_perm_gates(
            np.asarray(bih1) + np.asarray(bhh1))[None, :])
    if cfg.has_b2:
        shared["bt2"] = bf(_perm_gates(
            np.asarray(bih2) + np.asarray(bhh2))[None, :])

    src = np.asarray(src)
    src2 = (src // npc) * npad + (src % npc)  # remap into padded table2 rows

    in_maps = []
    for c in range(C):
        lo = c * npc
        xme = np.zeros((f, npad), NP_BF16)
        xme[:, :npc] = bf(x[lo:lo + npc].T)

        def pack(s):
            # [128, d*mt] with [p, t*mt+m] = s[m*128+p, t] for this core
            a = np.zeros((npad, d), np.int32)
            a[:npc] = s[lo:lo + npc]
            return np.ascontiguousarray(
                a.reshape(mt, 128, d).transpose(1, 2, 0).reshape(128, d * mt))

        m = dict(shared)
        m["xTme"] = xme
        m["idx1"] = pack(src.astype(np.int64))
        m["idx2"] = pack(src2.astype(np.int64))
        in_maps.append(m)
    return in_maps


def run(inputs: dict, trace: bool = False):
    """Build (cached), run on 8 cores, return (output [N,F] fp32, results)."""
    x = np.asarray(inputs["x"])
    n, f = x.shape
    d = np.asarray(inputs["src"]).shape[1]
    cfg = Cfg(
        n=n, d=d, f=f, cores=8,
        has_b1=bool(np.any(inputs["bih1"]) or np.any(inputs["bhh1"])),
        has_b2=bool(np.any(inputs["bih2"]) or np.any(inputs["bhh2"])),
    )
    if cfg not in _CACHE:
        _CACHE[cfg] = build(cfg)
    nc = _CACHE[cfg]
    in_maps = _prep_inputs(
        cfg, x, inputs["src"], inputs["Wih1"], inputs["Whh1"], inputs["bih1"],
        inputs["bhh1"], inputs["W_self"], inputs["W_neigh"], inputs["b"],
        inputs["Wih2"], inputs["Whh2"], inputs["bih2"], inputs["bhh2"],
    )
    res = run_bass_kernel_spmd(nc, in_maps, core_ids=list(range(cfg.cores)),
                               trace=trace)
    outp = np.concatenate([res.results[c]["out"] for c in range(cfg.cores)], 0)
    return np.ascontiguousarray(outp[:n], dtype=np.float32), res


def kernel(**inputs) -> np.ndarray:
    out, _ = run(inputs, trace=False)
    return out


# revision 12
# speedup vs baseline: 1.0544x; 1.0544x over previous
"""Trainium2 Bass kernel for a CustomSAGEConv (two LSTM-reduced GNN stages).

Computation (N nodes, D neighbors each, F features, H = F):
    mail1   = x[src]                          # [N, D, F]
    h_neigh = LSTM1(mail1).h_final            # [N, F]
    h       = x @ W_self + h_neigh @ W_neigh + b
    mail2   = h[src]
    out     = LSTM2(mail2).h_final            # [N, F]

Strategy (8 NeuronCores, dst-node sharding, weights replicated):
  * The LSTM input projection of a gathered mailbox equals a gather of the
    projected node table: (x[src]) @ Wih.T == (x @ Wih.T)[src].  Each core
    computes the projected table once (21 GFLOP instead of 671 GFLOP)
    and row-gathers [128, 4F] tiles per step with indirect DMA.
  * Recurrent h @ Whh.T runs per-core on the PE in bf16; hidden state is
    re-transposed each step with DMA transposes so it can be the stationary
    operand of the next step.
  * Gates are reordered host-side to [i, f, o, g] so one sigmoid covers
    [0:3F] and one tanh covers [3F:4F] on the scalar (ACT) engine.
  * Cell state c stays fp32; gates accumulate in fp32 PSUM; two of the
    cell muls run on GpSimd to unload the vector engine.
  * Stage-2 projection is node-sharded and AllGathered as a table.

kernel(**inputs) takes full-size numpy inputs and returns the full [N, F]
float32 output.
"""

import math
from contextlib import ExitStack
from dataclasses import dataclass

import ml_dtypes
import numpy as np

import concourse.bass as bass
import concourse.mybir as mybir
import concourse.tile as tile
from concourse import bacc
from concourse.bass_utils import run_bass_kernel_spmd
from concourse.masks import make_identity

BF16 = mybir.dt.bfloat16
F32 = mybir.dt.float32
I32 = mybir.dt.int32
AF = mybir.ActivationFunctionType
NP_BF16 = ml_dtypes.bfloat16


@dataclass(frozen=True)
class Cfg:
    n: int = 10000      # nodes
    d: int = 32         # in-degree (LSTM steps)
    f: int = 512        # features == hidden
    cores: int = 8
    has_b1: bool = False  # nonzero bih1+bhh1
    has_b2: bool = False  # nonzero bih2+bhh2

    @property
    def g(self):  # gate width
        return 4 * self.f

    @property
    def kt(self):  # contraction tiles of 128 over f
        return self.f // 128

    @property
    def npc(self):  # nodes per core
        return self.n // self.cores

    @property
    def mt(self):  # node tiles of 128 per core
        return math.ceil(self.npc / 128)

    @property
    def npad(self):  # padded nodes per core
        return self.mt * 128

    @property
    def mt_all(self):  # node tiles over all nodes (stage-1 table)
        return math.ceil(self.n / 128)

    @property
    def npad_all(self):
        return self.mt_all * 128


def _load_wtile(nc, pool, dram, kt, width, dtype=BF16, name=None):
    """DRAM [kt*128, width] -> SBUF [128, kt*width]; slice k at [:, k*width:+width]."""
    t = pool.tile([128, kt * width], dtype, tag=name, name=name or "wt")
    for k in range(kt):
        nc.sync.dma_start(
            out=t[:, k * width:(k + 1) * width],
            in_=dram[k * 128:(k + 1) * 128, :],
        )
    return t


def _proj_phase(nc, tc, cfg, stat_src, wt_sb, table, mtiles, bias_sb):
    """table[m*128+p, :] = sum_k stat(m,k).T @ w[k]  (+ ones.T @ bias).

    stat_src(m, k) -> AP [128, 128]: DRAM (DMA'd to SBUF here) or SBUF
    (used as the stationary operand directly).
    """
    g = cfg.g
    kt = cfg.kt
    with (
        tc.tile_pool(name="proj_stat", bufs=4) as statp,
        tc.tile_pool(name="proj_psum", bufs=2, space="PSUM") as psump,
        tc.tile_pool(name="proj_out", bufs=3) as outp,
    ):
        ones_sb = None
        if bias_sb is not None:
            ones_sb = statp.tile([1, 128], BF16, tag="ones", name="ones")
            nc.gpsimd.memset(ones_sb[:], 1.0)
        for m in range(mtiles):
            ps = psump.tile([128, g], F32, tag="pj", name="pj")
            for k in range(kt):
                src = stat_src(m, k)
                if src.space == bass.MemorySpace.DRAM:
                    st = statp.tile([128, 128], BF16, tag="st", name="st")
                    nc.sync.dma_start(out=st[:], in_=src)
                    src = st[:]
                for ns in range(g // 512):
                    nc.tensor.matmul(
                        ps[:, ns * 512:(ns + 1) * 512],
                        lhsT=src,
                        rhs=wt_sb[:, k * g + ns * 512: k * g + (ns + 1) * 512],
                        start=(k == 0),
                        stop=(k == kt - 1 and bias_sb is None),
                    )
            if bias_sb is not None:
                for ns in range(g // 512):
                    nc.tensor.matmul(
                        ps[:, ns * 512:(ns + 1) * 512],
                        lhsT=ones_sb[:],
                        rhs=bias_sb[:, ns * 512:(ns + 1) * 512],
                        start=False,
                        stop=True,
                    )
            ot = outp.tile([128, g], BF16, tag="pt", name="pt")
            nc.scalar.copy(ot[:], ps[:])
            nc.sync.dma_start(out=table[m * 128:(m + 1) * 128, :], in_=ot[:])


def _lstm_phase(nc, tc, cfg, table, idx_sb, whh_sb, ident, hT_sb, out_dram):
    """Run cfg.d LSTM steps over this core's cfg.mt node tiles.

    Gate layout is [i, f, o, g].  Hidden state is PE-transposed into hT_sb
    [128, kt*npad] (bf16); if out_dram is given the final hidden state is
    instead written there as fp32 [npc, f].
    """
    f, g, kt, mt, d, npad, npc = cfg.f, cfg.g, cfg.kt, cfg.mt, cfg.d, cfg.npad, cfg.npc
    halves = max(1, g // 1024)
    hw = min(g, 1024)
    hT_v = hT_sb[:].rearrange("p (k n) -> p k n", k=kt)

    with (
        tc.tile_pool(name="lstm_state", bufs=1) as cstp,
        tc.tile_pool(name="lstm_gather", bufs=6) as gp,
        tc.tile_pool(name="lstm_gates", bufs=3) as sp,
        tc.tile_pool(name="lstm_act", bufs=4) as ap_,
        tc.tile_pool(name="lstm_dve", bufs=4) as dp,
        tc.tile_pool(name="lstm_psum", bufs=3, space="PSUM") as pp,
        tc.tile_pool(name="lstm_tpsum", bufs=2, space="PSUM") as tpp,
    ):
        c_st = [cstp.tile([128, f], F32, tag=f"c{m}", name=f"c{m}") for m in range(mt)]
        for t in range(d):
            last = t == d - 1
            for m in range(mt):
                gsb = gp.tile([128, g], BF16, tag="g", name="g")
                nc.gpsimd.indirect_dma_start(
                    out=gsb[:],
                    out_offset=None,
                    in_=table[:, :],
                    in_offset=bass.IndirectOffsetOnAxis(
                        ap=idx_sb[:, t * mt + m: t * mt + m + 1], axis=0
                    ),
                )
                if t > 0:
                    gates = sp.tile([128, g], F32, tag="gt", name="gt")
                    for h in range(halves):
                        ps = pp.tile([128, hw], F32, tag="ps", name="ps")
                        for k in range(kt):
                            lhs = hT_sb[:, k * npad + m * 128: k * npad + (m + 1) * 128]
                            for ns in range(hw // 512):
                                c0 = h * hw + ns * 512
                                nc.tensor.matmul(
                                    ps[:, ns * 512:(ns + 1) * 512],
                                    lhsT=lhs,
                                    rhs=whh_sb[:, k * g + c0: k * g + c0 + 512],
                                    start=(k == 0),
                                    stop=(k == kt - 1),
                                )
                        nc.vector.tensor_add(
                            gates[:, h * hw:(h + 1) * hw], ps[:],
                            gsb[:, h * hw:(h + 1) * hw]
                        )
                else:
                    gates = gsb
                # gate order [i, f, o, g]: one sigmoid over 3F, one tanh
                sif = ap_.tile([128, 3 * f], BF16, tag="sif", name="sif")
                tg = ap_.tile([128, f], BF16, tag="tg", name="tg")
                nc.scalar.activation(sif[:], gates[:, 0:3 * f], AF.Sigmoid)
                nc.scalar.activation(tg[:], gates[:, 3 * f:4 * f], AF.Tanh)
                si, sf, so = sif[:, 0:f], sif[:, f:2 * f], sif[:, 2 * f:3 * f]
                if t > 0:
                    t1 = dp.tile([128, f], BF16, tag="t1", name="t1")
                    t2 = dp.tile([128, f], F32, tag="t2", name="t2")
                    nc.gpsimd.tensor_mul(t1[:], si, tg[:])
                    nc.vector.tensor_mul(t2[:], sf, c_st[m][:])
                    nc.vector.tensor_add(c_st[m][:], t1[:], t2[:])
                else:
                    nc.vector.tensor_mul(c_st[m][:], si, tg[:])
                tch = ap_.tile([128, f], BF16, tag="tc", name="tc")
                nc.scalar.activation(tch[:], c_st[m][:], AF.Tanh)
                if last and out_dram is not None:
                    hf = dp.tile([128, f], F32, tag="hf", name="hf")
                    nc.vector.tensor_mul(hf[:], so, tch[:])
                    rows = min(128, npc - m * 128)
                    nc.sync.dma_start(
                        out=out_dram[m * 128: m * 128 + rows, :], in_=hf[:rows, :]
                    )
                else:
                    hm = dp.tile([128, f], BF16, tag="hm", name="hm")
                    nc.gpsimd.tensor_mul(hm[:], so, tch[:])
                    pt = tpp.tile([128, f], BF16, tag="tp", name="tp")
                    for k in range(kt):
                        nc.tensor.transpose(
                            pt[:, k * 128:(k + 1) * 128],
                            hm[:, k * 128:(k + 1) * 128], ident[:]
                        )
                    nc.scalar.copy(
                        hT_v[:, :, m * 128:(m + 1) * 128],
                        pt[:].rearrange("p (k n) -> p k n", k=kt),
                    )


def build(cfg: Cfg):
    nc = bacc.Bacc("TRN2", target_bir_lowering=False, debug=False,
                   num_devices=cfg.cores)
    f, g, kt, mt, d, npad, npc, C = (
        cfg.f, cfg.g, cfg.kt, cfg.mt, cfg.d, cfg.npad, cfg.npc, cfg.cores
    )

    # --- I/O -------------------------------------------------------------
    xT = nc.dram_tensor("xT", [f, cfg.npad_all], BF16, kind="ExternalInput")
    xTme = nc.dram_tensor("xTme", [f, npad], BF16, kind="ExternalInput")
    wih1T = nc.dram_tensor("wih1T", [f, g], BF16, kind="ExternalInput")
    whh1T = nc.dram_tensor("whh1T", [f, g], BF16, kind="ExternalInput")
    wih2T = nc.dram_tensor("wih2T", [f, g], BF16, kind="ExternalInput")
    whh2T = nc.dram_tensor("whh2T", [f, g], BF16, kind="ExternalInput")
    wself = nc.dram_tensor("wself", [f, f], BF16, kind="ExternalInput")
    wneigh = nc.dram_tensor("wneigh", [f, f], BF16, kind="ExternalInput")
    blinT = nc.dram_tensor("blinT", [128, kt], F32, kind="ExternalInput")
    idx1 = nc.dram_tensor("idx1", [128, d * mt], I32, kind="ExternalInput")
    idx2 = nc.dram_tensor("idx2", [128, d * mt], I32, kind="ExternalInput")
    bt1 = bt2 = None
    if cfg.has_b1:
        bt1 = nc.dram_tensor("bt1", [1, g], BF16, kind="ExternalInput")
    if cfg.has_b2:
        bt2 = nc.dram_tensor("bt2", [1, g], BF16, kind="ExternalInput")
    out = nc.dram_tensor("out", [npc, f], F32, kind="ExternalOutput")

    # --- internal DRAM ---------------------------------------------------
    table1 = nc.dram_tensor("table1", [cfg.npad_all, g], BF16)
    nq = 4 if mt % 4 == 0 else 1  # table2 AllGather chunks
    qrows = npad // nq
    table2loc = nc.dram_tensor("table2loc", [npad, g], BF16)
    table2 = nc.dram_tensor("table2", [nq * C * qrows, g], BF16,
                            addr_space="Shared")

    with tile.TileContext(nc) as tc, ExitStack() as ctx:
        const = ctx.enter_context(tc.tile_pool(name="const", bufs=1))
        ident = const.tile([128, 128], BF16, tag="ident", name="ident")
        make_identity(nc, ident[:])
        idx1_sb = const.tile([128, d * mt], I32, tag="idx1", name="idx1")
        nc.sync.dma_start(out=idx1_sb[:], in_=idx1[:, :])
        idx2_sb = const.tile([128, d * mt], I32, tag="idx2", name="idx2")
        nc.sync.dma_start(out=idx2_sb[:], in_=idx2[:, :])
        blin_sb = const.tile([128, kt], F32, tag="blin", name="blin")
        nc.sync.dma_start(out=blin_sb[:], in_=blinT[:, :])

        # stage-1 hidden-state (transposed) persists into the linear stage
        s1pool = ctx.enter_context(tc.tile_pool(name="s1", bufs=1))
        hT1 = s1pool.tile([128, kt * npad], BF16, tag="hT1", name="hT1")

        # ---- stage 1: table1 = x @ Wih1.T (+b1) -------------------------
        with tc.tile_pool(name="w1", bufs=1) as w1p:
            wih1_sb = _load_wtile(nc, w1p, wih1T, kt, g, name="wih1")
            b1_sb = None
            if bt1 is not None:
                b1_sb = w1p.tile([1, g], BF16, tag="b1", name="b1")
                nc.sync.dma_start(out=b1_sb[:], in_=bt1[:, :])
            _proj_phase(
                nc, tc, cfg,
                lambda m, k: xT[k * 128:(k + 1) * 128, m * 128:(m + 1) * 128],
                wih1_sb, table1, cfg.mt_all, b1_sb,
            )

        # ---- stage 1: LSTM over mailboxes -------------------------------
        with tc.tile_pool(name="whh1", bufs=1) as whh1p:
            whh1_sb = _load_wtile(nc, whh1p, whh1T, kt, g, name="whh1")
            _lstm_phase(nc, tc, cfg, table1, idx1_sb, whh1_sb, ident, hT1, None)

        # ---- stage 1 linear (transposed) + local stage-2 projection -----
        with tc.tile_pool(name="lin", bufs=1) as linp:
            hlin_sb = linp.tile([128, kt * npad], BF16, tag="hlin", name="hlin")
            with (
                tc.tile_pool(name="lin_w", bufs=1) as linwp,
                tc.tile_pool(name="lin_psum", bufs=2, space="PSUM") as linpp,
            ):
                wself_sb = _load_wtile(nc, linwp, wself, kt, f, name="ws")
                wneigh_sb = _load_wtile(nc, linwp, wneigh, kt, f, name="wn")
                xtme_sb = _load_wtile(nc, linwp, xTme, kt, npad, name="xtme")
                nch = math.ceil(npad / 512)
                for hk in range(kt):
                    for nc_i in range(nch):
                        nw = min(512, npad - nc_i * 512)
                        ps = linpp.tile([128, 512], F32, tag="lp", name="lp")
                        for k in range(kt):
                            nc.tensor.matmul(
                                ps[:, :nw],
                                lhsT=wself_sb[:, k * f + hk * 128: k * f + hk * 128 + 128],
                                rhs=xtme_sb[:, k * npad + nc_i * 512: k * npad + nc_i * 512 + nw],
                                start=(k == 0),
                                stop=False,
                            )
                        for k in range(kt):
                            nc.tensor.matmul(
                                ps[:, :nw],
                                lhsT=wneigh_sb[:, k * f + hk * 128: k * f + hk * 128 + 128],
                                rhs=hT1[:, k * npad + nc_i * 512: k * npad + nc_i * 512 + nw],
                                start=False,
                                stop=(k == kt - 1),
                            )
                        nc.scalar.activation(
                            hlin_sb[:, hk * npad + nc_i * 512: hk * npad + nc_i * 512 + nw],
                            ps[:, :nw], AF.Identity,
                            bias=blin_sb[:, hk:hk + 1],
                        )

            # stage-2 projection of the LOCAL node block (stationary from SBUF)
            with tc.tile_pool(name="w2", bufs=1) as w2p:
                wih2_sb = _load_wtile(nc, w2p, wih2T, kt, g, name="wih2")
                b2_sb = None
                if bt2 is not None:
                    b2_sb = w2p.tile([1, g], BF16, tag="b2", name="b2")
                    nc.sync.dma_start(out=b2_sb[:], in_=bt2[:, :])
                _proj_phase(
                    nc, tc, cfg,
                    lambda m, k: hlin_sb[:, k * npad + m * 128:
                                         k * npad + (m + 1) * 128],
                    wih2_sb, table2loc, mt, b2_sb,
                )

        # ---- all-gather the stage-2 table (chunked, overlaps proj tail) --
        for q in range(nq):
            nc.gpsimd.collective_compute(
                "AllGather",
                mybir.AluOpType.bypass,
                replica_groups=[list(range(C))],
                ins=[table2loc[q * qrows:(q + 1) * qrows, :]],
                outs=[table2[q * C * qrows:(q + 1) * C * qrows, :]],
            )

        # ---- stage 2: LSTM -> out ---------------------------------------
        with (
            tc.tile_pool(name="whh2", bufs=1) as whh2p,
            tc.tile_pool(name="s2", bufs=1) as s2pool,
        ):
            whh2_sb = _load_wtile(nc, whh2p, whh2T, kt, g, name="whh2")
            hT2 = s2pool.tile([128, kt * npad], BF16, tag="hT2", name="hT2")
            _lstm_phase(nc, tc, cfg, table2, idx2_sb, whh2_sb, ident, hT2, out)

    nc.compile()
    return nc


_CACHE: dict = {}


def _perm_gates(w):
    """Reorder gate blocks [i, f, g, o] -> [i, f, o, g] along axis 0."""
    i, f_, g, o = np.split(np.asarray(w), 4, 0)
    return np.concatenate([i, f_, o, g], 0)


def _prep_inputs(cfg: Cfg, x, src, Wih1, Whh1, bih1, bhh1, W_self, W_neigh, b,
                 Wih2, Whh2, bih2, bhh2):
    """Build the 8 per-core input maps (host-side slicing/transposition only)."""
    f, g, d, mt, npc, npad, C = cfg.f, cfg.g, cfg.d, cfg.mt, cfg.npc, cfg.npad, cfg.cores

    def bf(a):
        return np.ascontiguousarray(a, dtype=np.float32).astype(NP_BF16)

    xT = np.zeros((f, cfg.npad_all), NP_BF16)
    xT[:, :cfg.n] = bf(x.T)
    shared = {
        "xT": xT,
        "wih1T": bf(_perm_gates(Wih1).T), "whh1T": bf(_perm_gates(Whh1).T),
        "wih2T": bf(_perm_gates(Wih2).T), "whh2T": bf(_perm_gates(Whh2).T),
        "wself": bf(W_self), "wneigh": bf(W_neigh),
        "blinT": np.ascontiguousarray(
            np.asarray(b, np.float32).reshape(cfg.kt, 128).T),
    }
    if cfg.has_b1:
        shared["bt1"] = bf(_perm_gates(
            np.asarray(bih1) + np.asarray(bhh1))[None, :])
    if cfg.has_b2:
        shared["bt2"] = bf(_perm_gates(
            np.asarray(bih2) + np.asarray(bhh2))[None, :])

    src = np.asarray(src)
    nq = 4 if mt % 4 == 0 else 1
    qrows = npad // nq
    c_ = src // npc
    r_ = src % npc  # local row in the owner's padded block (< npc <= npad)
    src2 = (r_ // qrows) * (C * qrows) + c_ * qrows + (r_ % qrows)

    in_maps = []
    for c in range(C):
        lo = c * npc
        xme = np.zeros((f, npad), NP_BF16)
        xme[:, :npc] = bf(x[lo:lo + npc].T)

        def pack(s):
            # [128, d*mt] with [p, t*mt+m] = s[m*128+p, t] for this core
            a = np.zeros((npad, d), np.int32)
            a[:npc] = s[lo:lo + npc]
            return np.ascontiguousarray(
                a.reshape(mt, 128, d).transpose(1, 2, 0).reshape(128, d * mt))

        m = dict(shared)
        m["xTme"] = xme
        m["idx1"] = pack(src.astype(np.int64))
        m["idx2"] = pack(src2.astype(np.int64))
        in_maps.append(m)
    return in_maps


def run(inputs: dict, trace: bool = False):
    """Build (cached), run on 8 cores, return (output [N,F] fp32, results)."""
    x = np.asarray(inputs["x"])
    n, f = x.shape
    d = np.asarray(inputs["src"]).shape[1]
    cfg = Cfg(
        n=n, d=d, f=f, cores=8,
        has_b1=bool(np.any(inputs["bih1"]) or np.any(inputs["bhh1"])),
        has_b2=bool(np.any(inputs["bih2"]) or np.any(inputs["bhh2"])),
    )
    if cfg not in _CACHE:
        _CACHE[cfg] = build(cfg)
    nc = _CACHE[cfg]
    in_maps = _prep_inputs(
        cfg, x, inputs["src"], inputs["Wih1"], inputs["Whh1"], inputs["bih1"],
        inputs["bhh1"], inputs["W_self"], inputs["W_neigh"], inputs["b"],
        inputs["Wih2"], inputs["Whh2"], inputs["bih2"], inputs["bhh2"],
    )
    res = run_bass_kernel_spmd(nc, in_maps, core_ids=list(range(cfg.cores)),
                               trace=trace)
    outp = np.concatenate([res.results[c]["out"] for c in range(cfg.cores)], 0)
    return np.ascontiguousarray(outp[:n], dtype=np.float32), res


def kernel(**inputs) -> np.ndarray:
    out, _ = run(inputs, trace=False)
    return out


# revision 13
# speedup vs baseline: 1.0797x; 1.0240x over previous
"""Trainium2 Bass kernel for a CustomSAGEConv (two LSTM-reduced GNN stages).

Computation (N nodes, D neighbors each, F features, H = F):
    mail1   = x[src]                          # [N, D, F]
    h_neigh = LSTM1(mail1).h_final            # [N, F]
    h       = x @ W_self + h_neigh @ W_neigh + b
    mail2   = h[src]
    out     = LSTM2(mail2).h_final            # [N, F]

Strategy (8 NeuronCores, dst-node sharding, weights replicated):
  * The LSTM input projection of a gathered mailbox equals a gather of the
    projected node table: (x[src]) @ Wih.T == (x @ Wih.T)[src].  Each core
    computes the projected table once (21 GFLOP instead of 671 GFLOP)
    and row-gathers [128, 4F] tiles per step with indirect DMA.
  * Recurrent h @ Whh.T runs per-core on the PE in bf16; hidden state is
    re-transposed each step with DMA transposes so it can be the stationary
    operand of the next step.
  * Gates are reordered host-side to [i, f, o, g] so one sigmoid covers
    [0:3F] and one tanh covers [3F:4F] on the scalar (ACT) engine.
  * Cell state c stays fp32; gates accumulate in fp32 PSUM; two of the
    cell muls run on GpSimd to unload the vector engine.
  * Stage-2 projection is node-sharded and AllGathered as a table.

kernel(**inputs) takes full-size numpy inputs and returns the full [N, F]
float32 output.
"""

import math
from contextlib import ExitStack
from dataclasses import dataclass

import ml_dtypes
import numpy as np

import concourse.bass as bass
import concourse.mybir as mybir
import concourse.tile as tile
from concourse import bacc
from concourse.bass_utils import run_bass_kernel_spmd
from concourse.masks import make_identity

BF16 = mybir.dt.bfloat16
F32 = mybir.dt.float32
I32 = mybir.dt.int32
AF = mybir.ActivationFunctionType
NP_BF16 = ml_dtypes.bfloat16


@dataclass(frozen=True)
class Cfg:
    n: int = 10000      # nodes
    d: int = 32         # in-degree (LSTM steps)
    f: int = 512        # features == hidden
    cores: int = 8
    has_b1: bool = False  # nonzero bih1+bhh1
    has_b2: bool = False  # nonzero bih2+bhh2

    @property
    def g(self):  # gate width
        return 4 * self.f

    @property
    def kt(self):  # contraction tiles of 128 over f
        return self.f // 128

    @property
    def npc(self):  # nodes per core
        return self.n // self.cores

    @property
    def mt(self):  # node tiles of 128 per core
        return math.ceil(self.npc / 128)

    @property
    def npad(self):  # padded nodes per core
        return self.mt * 128

    @property
    def mt_all(self):  # node tiles over all nodes (stage-1 table)
        return math.ceil(self.n / 128)

    @property
    def npad_all(self):
        return self.mt_all * 128


def _load_wtile(nc, pool, dram, kt, width, dtype=BF16, name=None):
    """DRAM [kt*128, width] -> SBUF [128, kt*width]; slice k at [:, k*width:+width]."""
    t = pool.tile([128, kt * width], dtype, tag=name, name=name or "wt")
    for k in range(kt):
        nc.sync.dma_start(
            out=t[:, k * width:(k + 1) * width],
            in_=dram[k * 128:(k + 1) * 128, :],
        )
    return t


def _proj_phase(nc, tc, cfg, stat_src, wt_sb, table, mtiles, bias_sb):
    """table[m*128+p, :] = sum_k stat(m,k).T @ w[k]  (+ ones.T @ bias).

    stat_src(m, k) -> AP [128, 128]: DRAM (DMA'd to SBUF here) or SBUF
    (used as the stationary operand directly).
    """
    g = cfg.g
    kt = cfg.kt
    with (
        tc.tile_pool(name="proj_stat", bufs=8) as statp,
        tc.tile_pool(name="proj_psum", bufs=2, space="PSUM") as psump,
        tc.tile_pool(name="proj_out", bufs=4) as outp,
    ):
        ones_sb = None
        if bias_sb is not None:
            ones_sb = statp.tile([1, 128], BF16, tag="ones", name="ones")
            nc.gpsimd.memset(ones_sb[:], 1.0)
        for m in range(mtiles):
            ps = psump.tile([128, g], F32, tag="pj", name="pj")
            for k in range(kt):
                src = stat_src(m, k)
                if src.space == bass.MemorySpace.DRAM:
                    st = statp.tile([128, 128], BF16, tag="st", name="st")
                    nc.sync.dma_start(out=st[:], in_=src)
                    src = st[:]
                for ns in range(g // 512):
                    nc.tensor.matmul(
                        ps[:, ns * 512:(ns + 1) * 512],
                        lhsT=src,
                        rhs=wt_sb[:, k * g + ns * 512: k * g + (ns + 1) * 512],
                        start=(k == 0),
                        stop=(k == kt - 1 and bias_sb is None),
                    )
            if bias_sb is not None:
                for ns in range(g // 512):
                    nc.tensor.matmul(
                        ps[:, ns * 512:(ns + 1) * 512],
                        lhsT=ones_sb[:],
                        rhs=bias_sb[:, ns * 512:(ns + 1) * 512],
                        start=False,
                        stop=True,
                    )
            ot = outp.tile([128, g], BF16, tag="pt", name="pt")
            nc.scalar.copy(ot[:], ps[:])
            nc.sync.dma_start(out=table[m * 128:(m + 1) * 128, :], in_=ot[:])


def _lstm_phase(nc, tc, cfg, table, idx_sb, whh_sb, ident, hT_sb, out_dram):
    """Run cfg.d LSTM steps over this core's cfg.mt node tiles.

    Gate layout is [i, f, o, g].  Hidden state is PE-transposed into hT_sb
    [128, kt*npad] (bf16); if out_dram is given the final hidden state is
    instead written there as fp32 [npc, f].
    """
    f, g, kt, mt, d, npad, npc = cfg.f, cfg.g, cfg.kt, cfg.mt, cfg.d, cfg.npad, cfg.npc
    halves = max(1, g // 1024)
    hw = min(g, 1024)
    hT_v = hT_sb[:].rearrange("p (k n) -> p k n", k=kt)

    with (
        tc.tile_pool(name="lstm_state", bufs=1) as cstp,
        tc.tile_pool(name="lstm_gather", bufs=8) as gp,
        tc.tile_pool(name="lstm_gates", bufs=4) as sp,
        tc.tile_pool(name="lstm_act", bufs=6) as ap_,
        tc.tile_pool(name="lstm_dve", bufs=6) as dp,
        tc.tile_pool(name="lstm_psum", bufs=3, space="PSUM") as pp,
        tc.tile_pool(name="lstm_tpsum", bufs=2, space="PSUM") as tpp,
    ):
        c_st = [cstp.tile([128, f], F32, tag=f"c{m}", name=f"c{m}") for m in range(mt)]
        for t in range(d):
            last = t == d - 1
            for m in range(mt):
                gsb = gp.tile([128, g], BF16, tag="g", name="g")
                nc.gpsimd.indirect_dma_start(
                    out=gsb[:],
                    out_offset=None,
                    in_=table[:, :],
                    in_offset=bass.IndirectOffsetOnAxis(
                        ap=idx_sb[:, t * mt + m: t * mt + m + 1], axis=0
                    ),
                )
                if t > 0:
                    gates = sp.tile([128, g], F32, tag="gt", name="gt")
                    for h in range(halves):
                        ps = pp.tile([128, hw], F32, tag="ps", name="ps")
                        for k in range(kt):
                            lhs = hT_sb[:, k * npad + m * 128: k * npad + (m + 1) * 128]
                            for ns in range(hw // 512):
                                c0 = h * hw + ns * 512
                                nc.tensor.matmul(
                                    ps[:, ns * 512:(ns + 1) * 512],
                                    lhsT=lhs,
                                    rhs=whh_sb[:, k * g + c0: k * g + c0 + 512],
                                    start=(k == 0),
                                    stop=(k == kt - 1),
                                )
                        nc.vector.tensor_add(
                            gates[:, h * hw:(h + 1) * hw], ps[:],
                            gsb[:, h * hw:(h + 1) * hw]
                        )
                else:
                    gates = gsb
                # gate order [i, f, o, g]: one sigmoid over 3F, one tanh
                sif = ap_.tile([128, 3 * f], BF16, tag="sif", name="sif")
                tg = ap_.tile([128, f], BF16, tag="tg", name="tg")
                nc.scalar.activation(sif[:], gates[:, 0:3 * f], AF.Sigmoid)
                nc.scalar.activation(tg[:], gates[:, 3 * f:4 * f], AF.Tanh)
                si, sf, so = sif[:, 0:f], sif[:, f:2 * f], sif[:, 2 * f:3 * f]
                if t > 0:
                    t1 = dp.tile([128, f], BF16, tag="t1", name="t1")
                    t2 = dp.tile([128, f], F32, tag="t2", name="t2")
                    nc.gpsimd.tensor_mul(t1[:], si, tg[:])
                    nc.vector.tensor_mul(t2[:], sf, c_st[m][:])
                    nc.vector.tensor_add(c_st[m][:], t1[:], t2[:])
                else:
                    nc.vector.tensor_mul(c_st[m][:], si, tg[:])
                tch = ap_.tile([128, f], BF16, tag="tc", name="tc")
                nc.scalar.activation(tch[:], c_st[m][:], AF.Tanh)
                if last and out_dram is not None:
                    hf = dp.tile([128, f], F32, tag="hf", name="hf")
                    nc.vector.tensor_mul(hf[:], so, tch[:])
                    rows = min(128, npc - m * 128)
                    nc.sync.dma_start(
                        out=out_dram[m * 128: m * 128 + rows, :], in_=hf[:rows, :]
                    )
                else:
                    hm = dp.tile([128, f], BF16, tag="hm", name="hm")
                    nc.gpsimd.tensor_mul(hm[:], so, tch[:])
                    pt = tpp.tile([128, f], BF16, tag="tp", name="tp")
                    for k in range(kt):
                        nc.tensor.transpose(
                            pt[:, k * 128:(k + 1) * 128],
                            hm[:, k * 128:(k + 1) * 128], ident[:]
                        )
                    nc.scalar.copy(
                        hT_v[:, :, m * 128:(m + 1) * 128],
                        pt[:].rearrange("p (k n) -> p k n", k=kt),
                    )


def build(cfg: Cfg):
    nc = bacc.Bacc("TRN2", target_bir_lowering=False, debug=False,
                   num_devices=cfg.cores)
    f, g, kt, mt, d, npad, npc, C = (
        cfg.f, cfg.g, cfg.kt, cfg.mt, cfg.d, cfg.npad, cfg.npc, cfg.cores
    )

    # --- I/O -------------------------------------------------------------
    xT = nc.dram_tensor("xT", [f, cfg.npad_all], BF16, kind="ExternalInput")
    xTme = nc.dram_tensor("xTme", [f, npad], BF16, kind="ExternalInput")
    wih1T = nc.dram_tensor("wih1T", [f, g], BF16, kind="ExternalInput")
    whh1T = nc.dram_tensor("whh1T", [f, g], BF16, kind="ExternalInput")
    wih2T = nc.dram_tensor("wih2T", [f, g], BF16, kind="ExternalInput")
    whh2T = nc.dram_tensor("whh2T", [f, g], BF16, kind="ExternalInput")
    wself = nc.dram_tensor("wself", [f, f], BF16, kind="ExternalInput")
    wneigh = nc.dram_tensor("wneigh", [f, f], BF16, kind="ExternalInput")
    blinT = nc.dram_tensor("blinT", [128, kt], F32, kind="ExternalInput")
    idx1 = nc.dram_tensor("idx1", [128, d * mt], I32, kind="ExternalInput")
    idx2 = nc.dram_tensor("idx2", [128, d * mt], I32, kind="ExternalInput")
    bt1 = bt2 = None
    if cfg.has_b1:
        bt1 = nc.dram_tensor("bt1", [1, g], BF16, kind="ExternalInput")
    if cfg.has_b2:
        bt2 = nc.dram_tensor("bt2", [1, g], BF16, kind="ExternalInput")
    out = nc.dram_tensor("out", [npc, f], F32, kind="ExternalOutput")

    # --- internal DRAM ---------------------------------------------------
    table1 = nc.dram_tensor("table1", [cfg.npad_all, g], BF16)
    nq = 4 if mt % 4 == 0 else 1  # table2 AllGather chunks
    qrows = npad // nq
    table2loc = nc.dram_tensor("table2loc", [npad, g], BF16)
    table2 = nc.dram_tensor("table2", [nq * C * qrows, g], BF16,
                            addr_space="Shared")

    with tile.TileContext(nc) as tc, ExitStack() as ctx:
        const = ctx.enter_context(tc.tile_pool(name="const", bufs=1))
        ident = const.tile([128, 128], BF16, tag="ident", name="ident")
        make_identity(nc, ident[:])
        idx1_sb = const.tile([128, d * mt], I32, tag="idx1", name="idx1")
        nc.sync.dma_start(out=idx1_sb[:], in_=idx1[:, :])
        idx2_sb = const.tile([128, d * mt], I32, tag="idx2", name="idx2")
        nc.sync.dma_start(out=idx2_sb[:], in_=idx2[:, :])
        blin_sb = const.tile([128, kt], F32, tag="blin", name="blin")
        nc.sync.dma_start(out=blin_sb[:], in_=blinT[:, :])

        # stage-1 hidden-state (transposed) persists into the linear stage
        s1pool = ctx.enter_context(tc.tile_pool(name="s1", bufs=1))
        hT1 = s1pool.tile([128, kt * npad], BF16, tag="hT1", name="hT1")

        # ---- stage 1: table1 = x @ Wih1.T (+b1) -------------------------
        with tc.tile_pool(name="w1", bufs=1) as w1p:
            wih1_sb = _load_wtile(nc, w1p, wih1T, kt, g, name="wih1")
            b1_sb = None
            if bt1 is not None:
                b1_sb = w1p.tile([1, g], BF16, tag="b1", name="b1")
                nc.sync.dma_start(out=b1_sb[:], in_=bt1[:, :])
            _proj_phase(
                nc, tc, cfg,
                lambda m, k: xT[k * 128:(k + 1) * 128, m * 128:(m + 1) * 128],
                wih1_sb, table1, cfg.mt_all, b1_sb,
            )

        # ---- stage 1: LSTM over mailboxes -------------------------------
        with tc.tile_pool(name="whh1", bufs=1) as whh1p:
            whh1_sb = _load_wtile(nc, whh1p, whh1T, kt, g, name="whh1")
            _lstm_phase(nc, tc, cfg, table1, idx1_sb, whh1_sb, ident, hT1, None)

        # ---- stage 1 linear (transposed) + local stage-2 projection -----
        with tc.tile_pool(name="lin", bufs=1) as linp:
            hlin_sb = linp.tile([128, kt * npad], BF16, tag="hlin", name="hlin")
            with (
                tc.tile_pool(name="lin_w", bufs=1) as linwp,
                tc.tile_pool(name="lin_psum", bufs=2, space="PSUM") as linpp,
            ):
                wself_sb = _load_wtile(nc, linwp, wself, kt, f, name="ws")
                wneigh_sb = _load_wtile(nc, linwp, wneigh, kt, f, name="wn")
                xtme_sb = _load_wtile(nc, linwp, xTme, kt, npad, name="xtme")
                nch = math.ceil(npad / 512)
                for hk in range(kt):
                    for nc_i in range(nch):
                        nw = min(512, npad - nc_i * 512)
                        ps = linpp.tile([128, 512], F32, tag="lp", name="lp")
                        for k in range(kt):
                            nc.tensor.matmul(
                                ps[:, :nw],
                                lhsT=wself_sb[:, k * f + hk * 128: k * f + hk * 128 + 128],
                                rhs=xtme_sb[:, k * npad + nc_i * 512: k * npad + nc_i * 512 + nw],
                                start=(k == 0),
                                stop=False,
                            )
                        for k in range(kt):
                            nc.tensor.matmul(
                                ps[:, :nw],
                                lhsT=wneigh_sb[:, k * f + hk * 128: k * f + hk * 128 + 128],
                                rhs=hT1[:, k * npad + nc_i * 512: k * npad + nc_i * 512 + nw],
                                start=False,
                                stop=(k == kt - 1),
                            )
                        nc.scalar.activation(
                            hlin_sb[:, hk * npad + nc_i * 512: hk * npad + nc_i * 512 + nw],
                            ps[:, :nw], AF.Identity,
                            bias=blin_sb[:, hk:hk + 1],
                        )

            # stage-2 projection of the LOCAL node block (stationary from SBUF)
            with tc.tile_pool(name="w2", bufs=1) as w2p:
                wih2_sb = _load_wtile(nc, w2p, wih2T, kt, g, name="wih2")
                b2_sb = None
                if bt2 is not None:
                    b2_sb = w2p.tile([1, g], BF16, tag="b2", name="b2")
                    nc.sync.dma_start(out=b2_sb[:], in_=bt2[:, :])
                _proj_phase(
                    nc, tc, cfg,
                    lambda m, k: hlin_sb[:, k * npad + m * 128:
                                         k * npad + (m + 1) * 128],
                    wih2_sb, table2loc, mt, b2_sb,
                )

        # ---- all-gather the stage-2 table (chunked, overlaps proj tail) --
        for q in range(nq):
            nc.gpsimd.collective_compute(
                "AllGather",
                mybir.AluOpType.bypass,
                replica_groups=[list(range(C))],
                ins=[table2loc[q * qrows:(q + 1) * qrows, :]],
                outs=[table2[q * C * qrows:(q + 1) * C * qrows, :]],
            )

        # ---- stage 2: LSTM -> out ---------------------------------------
        with (
            tc.tile_pool(name="whh2", bufs=1) as whh2p,
            tc.tile_pool(name="s2", bufs=1) as s2pool,
        ):
            whh2_sb = _load_wtile(nc, whh2p, whh2T, kt, g, name="whh2")
            hT2 = s2pool.tile([128, kt * npad], BF16, tag="hT2", name="hT2")
            _lstm_phase(nc, tc, cfg, table2, idx2_sb, whh2_sb, ident, hT2, out)

    nc.compile()
    return nc


_CACHE: dict = {}


def _perm_gates(w):
    """Reorder gate blocks [i, f, g, o] -> [i, f, o, g] along axis 0."""
    i, f_, g, o = np.split(np.asarray(w), 4, 0)
    return np.concatenate([i, f_, o, g], 0)


def _prep_inputs(cfg: Cfg, x, src, Wih1, Whh1, bih1, bhh1, W_self, W_neigh, b,
                 Wih2, Whh2, bih2, bhh2):
    """Build the 8 per-core input maps (host-side slicing/transposition only)."""
    f, g, d, mt, npc, npad, C = cfg.f, cfg.g, cfg.d, cfg.mt, cfg.npc, cfg.npad, cfg.cores

    def bf(a):
        return np.ascontiguousarray(a, dtype=np.float32).astype(NP_BF16)

    xT = np.zeros((f, cfg.npad_all), NP_BF16)
    xT[:, :cfg.n] = bf(x.T)
    shared = {
        "xT": xT,
        "wih1T": bf(_perm_gates(Wih1).T), "whh1T": bf(_perm_gates(Whh1).T),
        "wih2T": bf(_perm_gates(Wih2).T), "whh2T": bf(_perm_gates(Whh2).T),
        "wself": bf(W_self), "wneigh": bf(W_neigh),
        "blinT": np.ascontiguousarray(
            np.asarray(b, np.float32).reshape(cfg.kt, 128).T),
    }
    if cfg.has_b1:
        shared["bt1"] = bf(_perm_gates(
            np.asarray(bih1) + np.asarray(bhh1))[None, :])
    if cfg.has_b2:
        shared["bt2"] = bf(_perm_gates(
            np.asarray(bih2) + np.asarray(bhh2))[None, :])

    src = np.asarray(src)
    nq = 4 if mt % 4 == 0 else 1
    qrows = npad // nq
    c_ = src // npc
    r_ = src % npc  # local row in the owner's padded block (< npc <= npad)
    src2 = (r_ // qrows) * (C * qrows) + c_ * qrows + (r_ % qrows)

    in_maps = []
    for c in range(C):
        lo = c * npc
        xme = np.zeros((f, npad), NP_BF16)
        xme[:, :npc] = bf(x[lo:lo + npc].T)

        def pack(s):
            # [128, d*mt] with [p, t*mt+m] = s[m*128+p, t] for this core
            a = np.zeros((npad, d), np.int32)
            a[:npc] = s[lo:lo + npc]
            return np.ascontiguousarray(
                a.reshape(mt, 128, d).transpose(1, 2, 0).reshape(128, d * mt))

        m = dict(shared)
        m["xTme"] = xme
        m["idx1"] = pack(src.astype(np.int64))
        m["idx2"] = pack(src2.astype(np.int64))
        in_maps.append(m)
    return in_maps


def run(inputs: dict, trace: bool = False):
    """Build (cached), run on 8 cores, return (output [N,F] fp32, results)."""
    x = np.asarray(inputs["x"])
    n, f = x.shape
    d = np.asarray(inputs["src"]).shape[1]
    cfg = Cfg(
        n=n, d=d, f=f, cores=8,
        has_b1=bool(np.any(inputs["bih1"]) or np.any(inputs["bhh1"])),
        has_b2=bool(np.any(inputs["bih2"]) or np.any(inputs["bhh2"])),
    )
    if cfg not in _CACHE:
        _CACHE[cfg] = build(cfg)
    nc = _CACHE[cfg]
    in_maps = _prep_inputs(
        cfg, x, inputs["src"], inputs["Wih1"], inputs["Whh1"], inputs["bih1"],
        inputs["bhh1"], inputs["W_self"], inputs["W_neigh"], inputs["b"],
        inputs["Wih2"], inputs["Whh2"], inputs["bih2"], inputs["bhh2"],
    )
    res = run_bass_kernel_spmd(nc, in_maps, core_ids=list(range(cfg.cores)),
                               trace=trace)
    outp = np.concatenate([res.results[c]["out"] for c in range(cfg.cores)], 0)
    return np.ascontiguousarray(outp[:n], dtype=np.float32), res


def kernel(**inputs) -> np.ndarray:
    out, _ = run(inputs, trace=False)
    return out


# revision 14
# speedup vs baseline: 1.1184x; 1.0359x over previous
"""Trainium2 Bass kernel for a CustomSAGEConv (two LSTM-reduced GNN stages).

Computation (N nodes, D neighbors each, F features, H = F):
    mail1   = x[src]                          # [N, D, F]
    h_neigh = LSTM1(mail1).h_final            # [N, F]
    h       = x @ W_self + h_neigh @ W_neigh + b
    mail2   = h[src]
    out     = LSTM2(mail2).h_final            # [N, F]

Strategy (8 NeuronCores, dst-node sharding, weights replicated):
  * The LSTM input projection of a gathered mailbox equals a gather of the
    projected node table: (x[src]) @ Wih.T == (x @ Wih.T)[src].  Each core
    computes the projected table once (21 GFLOP instead of 671 GFLOP)
    and row-gathers [128, 4F] tiles per step with indirect DMA.
  * Recurrent h @ Whh.T runs per-core on the PE in bf16; hidden state is
    re-transposed each step with DMA transposes so it can be the stationary
    operand of the next step.
  * Gates are reordered host-side to [i, f, o, g] so one sigmoid covers
    [0:3F] and one tanh covers [3F:4F] on the scalar (ACT) engine.
  * Cell state c stays fp32; gates accumulate in fp32 PSUM; two of the
    cell muls run on GpSimd to unload the vector engine.
  * Stage-2 projection is node-sharded and AllGathered as a table.

kernel(**inputs) takes full-size numpy inputs and returns the full [N, F]
float32 output.
"""

import math
from contextlib import ExitStack
from dataclasses import dataclass

import ml_dtypes
import numpy as np

import concourse.bass as bass
import concourse.mybir as mybir
import concourse.tile as tile
from concourse import bacc
from concourse.bass_utils import run_bass_kernel_spmd
from concourse.masks import make_identity

BF16 = mybir.dt.bfloat16
F32 = mybir.dt.float32
I32 = mybir.dt.int32
AF = mybir.ActivationFunctionType
NP_BF16 = ml_dtypes.bfloat16


@dataclass(frozen=True)
class Cfg:
    n: int = 10000      # nodes
    d: int = 32         # in-degree (LSTM steps)
    f: int = 512        # features == hidden
    cores: int = 8
    has_b1: bool = False  # nonzero bih1+bhh1
    has_b2: bool = False  # nonzero bih2+bhh2

    @property
    def g(self):  # gate width
        return 4 * self.f

    @property
    def kt(self):  # contraction tiles of 128 over f
        return self.f // 128

    @property
    def npc(self):  # nodes per core
        return self.n // self.cores

    @property
    def mt(self):  # node tiles of 128 per core
        return math.ceil(self.npc / 128)

    @property
    def npad(self):  # padded nodes per core
        return self.mt * 128

    @property
    def mt_all(self):  # node tiles over all nodes (stage-1 table)
        return math.ceil(self.n / 128)

    @property
    def npad_all(self):
        return self.mt_all * 128


def _load_wtile(nc, pool, dram, kt, width, dtype=BF16, name=None):
    """DRAM [kt*128, width] -> SBUF [128, kt*width]; slice k at [:, k*width:+width]."""
    t = pool.tile([128, kt * width], dtype, tag=name, name=name or "wt")
    for k in range(kt):
        nc.sync.dma_start(
            out=t[:, k * width:(k + 1) * width],
            in_=dram[k * 128:(k + 1) * 128, :],
        )
    return t


def _proj_phase(nc, tc, cfg, stat_src, wt_sb, table, mtiles, bias_sb):
    """table[m*128+p, :] = sum_k stat(m,k).T @ w[k]  (+ ones.T @ bias).

    stat_src(m, k) -> AP [128, 128]: DRAM (DMA'd to SBUF here) or SBUF
    (used as the stationary operand directly).
    """
    g = cfg.g
    kt = cfg.kt
    with (
        tc.tile_pool(name="proj_stat", bufs=8) as statp,
        tc.tile_pool(name="proj_psum", bufs=2, space="PSUM") as psump,
        tc.tile_pool(name="proj_out", bufs=4) as outp,
    ):
        ones_sb = None
        if bias_sb is not None:
            ones_sb = statp.tile([1, 128], BF16, tag="ones", name="ones")
            nc.gpsimd.memset(ones_sb[:], 1.0)
        for m in range(mtiles):
            ps = psump.tile([128, g], F32, tag="pj", name="pj")
            for k in range(kt):
                src = stat_src(m, k)
                if src.space == bass.MemorySpace.DRAM:
                    st = statp.tile([128, 128], BF16, tag="st", name="st")
                    nc.sync.dma_start(out=st[:], in_=src)
                    src = st[:]
                for ns in range(g // 512):
                    nc.tensor.matmul(
                        ps[:, ns * 512:(ns + 1) * 512],
                        lhsT=src,
                        rhs=wt_sb[:, k * g + ns * 512: k * g + (ns + 1) * 512],
                        start=(k == 0),
                        stop=(k == kt - 1 and bias_sb is None),
                    )
            if bias_sb is not None:
                for ns in range(g // 512):
                    nc.tensor.matmul(
                        ps[:, ns * 512:(ns + 1) * 512],
                        lhsT=ones_sb[:],
                        rhs=bias_sb[:, ns * 512:(ns + 1) * 512],
                        start=False,
                        stop=True,
                    )
            ot = outp.tile([128, g], BF16, tag="pt", name="pt")
            nc.scalar.copy(ot[:], ps[:])
            nc.sync.dma_start(out=table[m * 128:(m + 1) * 128, :], in_=ot[:])


def _lstm_phase(nc, tc, cfg, table, idx_sb, whh_sb, ident, hT_sb, out_dram):
    """Run cfg.d LSTM steps over this core's cfg.mt node tiles.

    Gate layout is [i, f, o, g].  Hidden state is PE-transposed into hT_sb
    [128, kt*npad] (bf16); if out_dram is given the final hidden state is
    instead written there as fp32 [npc, f].
    """
    f, g, kt, mt, d, npad, npc = cfg.f, cfg.g, cfg.kt, cfg.mt, cfg.d, cfg.npad, cfg.npc
    halves = max(1, g // 1024)
    hw = min(g, 1024)
    hT_v = hT_sb[:].rearrange("p (k n) -> p k n", k=kt)

    with (
        tc.tile_pool(name="lstm_state", bufs=1) as cstp,
        tc.tile_pool(name="lstm_gather", bufs=8) as gp,
        tc.tile_pool(name="lstm_gates", bufs=4) as sp,
        tc.tile_pool(name="lstm_act", bufs=6) as ap_,
        tc.tile_pool(name="lstm_dve", bufs=6) as dp,
        tc.tile_pool(name="lstm_psum", bufs=3, space="PSUM") as pp,
        tc.tile_pool(name="lstm_tpsum", bufs=2, space="PSUM") as tpp,
    ):
        c_st = [cstp.tile([128, f], F32, tag=f"c{m}", name=f"c{m}") for m in range(mt)]
        for t in range(d):
            last = t == d - 1
            for m in range(mt):
                gsb = gp.tile([128, g], BF16, tag="g", name="g")
                nc.gpsimd.indirect_dma_start(
                    out=gsb[:],
                    out_offset=None,
                    in_=table[:, :],
                    in_offset=bass.IndirectOffsetOnAxis(
                        ap=idx_sb[:, t * mt + m: t * mt + m + 1], axis=0
                    ),
                )
                if t > 0:
                    gates = sp.tile([128, g], F32, tag="gt", name="gt")
                    for h in range(halves):
                        ps = pp.tile([128, hw], F32, tag="ps", name="ps")
                        for k in range(kt):
                            lhs = hT_sb[:, k * npad + m * 128: k * npad + (m + 1) * 128]
                            for ns in range(hw // 512):
                                c0 = h * hw + ns * 512
                                nc.tensor.matmul(
                                    ps[:, ns * 512:(ns + 1) * 512],
                                    lhsT=lhs,
                                    rhs=whh_sb[:, k * g + c0: k * g + c0 + 512],
                                    start=(k == 0),
                                    stop=(k == kt - 1),
                                )
                        nc.vector.tensor_add(
                            gates[:, h * hw:(h + 1) * hw], ps[:],
                            gsb[:, h * hw:(h + 1) * hw]
                        )
                else:
                    gates = gsb
                # gate order [i, f, o, g]: one sigmoid over 3F, one tanh
                sif = ap_.tile([128, 3 * f], BF16, tag="sif", name="sif")
                tg = ap_.tile([128, f], BF16, tag="tg", name="tg")
                nc.scalar.activation(sif[:], gates[:, 0:3 * f], AF.Sigmoid)
                nc.scalar.activation(tg[:], gates[:, 3 * f:4 * f], AF.Tanh)
                si, sf, so = sif[:, 0:f], sif[:, f:2 * f], sif[:, 2 * f:3 * f]
                if t > 0:
                    t1 = dp.tile([128, f], BF16, tag="t1", name="t1")
                    t2 = dp.tile([128, f], F32, tag="t2", name="t2")
                    nc.gpsimd.tensor_mul(t1[:], si, tg[:])
                    nc.vector.tensor_mul(t2[:], sf, c_st[m][:])
                    nc.vector.tensor_add(c_st[m][:], t1[:], t2[:])
                else:
                    nc.vector.tensor_mul(c_st[m][:], si, tg[:])
                tch = ap_.tile([128, f], BF16, tag="tc", name="tc")
                nc.scalar.activation(tch[:], c_st[m][:], AF.Tanh)
                if last and out_dram is not None:
                    hf = dp.tile([128, f], F32, tag="hf", name="hf")
                    nc.vector.tensor_mul(hf[:], so, tch[:])
                    rows = min(128, npc - m * 128)
                    nc.sync.dma_start(
                        out=out_dram[m * 128: m * 128 + rows, :], in_=hf[:rows, :]
                    )
                else:
                    hm = dp.tile([128, f], BF16, tag="hm", name="hm")
                    nc.gpsimd.tensor_mul(hm[:], so, tch[:])
                    pt = tpp.tile([128, f], BF16, tag="tp", name="tp")
                    for k in range(kt):
                        nc.tensor.transpose(
                            pt[:, k * 128:(k + 1) * 128],
                            hm[:, k * 128:(k + 1) * 128], ident[:]
                        )
                    nc.scalar.copy(
                        hT_v[:, :, m * 128:(m + 1) * 128],
                        pt[:].rearrange("p (k n) -> p k n", k=kt),
                    )


def build(cfg: Cfg):
    nc = bacc.Bacc("TRN2", target_bir_lowering=False, debug=False,
                   num_devices=cfg.cores)
    f, g, kt, mt, d, npad, npc, C = (
        cfg.f, cfg.g, cfg.kt, cfg.mt, cfg.d, cfg.npad, cfg.npc, cfg.cores
    )

    # --- I/O -------------------------------------------------------------
    xTloc = nc.dram_tensor("xTloc", [f, npad], BF16, kind="ExternalInput")
    xTme = nc.dram_tensor("xTme", [f, npad], BF16, kind="ExternalInput")
    wih1T = nc.dram_tensor("wih1T", [f, g], BF16, kind="ExternalInput")
    whh1T = nc.dram_tensor("whh1T", [f, g], BF16, kind="ExternalInput")
    wih2T = nc.dram_tensor("wih2T", [f, g], BF16, kind="ExternalInput")
    whh2T = nc.dram_tensor("whh2T", [f, g], BF16, kind="ExternalInput")
    wself = nc.dram_tensor("wself", [f, f], BF16, kind="ExternalInput")
    wneigh = nc.dram_tensor("wneigh", [f, f], BF16, kind="ExternalInput")
    blinT = nc.dram_tensor("blinT", [128, kt], F32, kind="ExternalInput")
    idx1 = nc.dram_tensor("idx1", [128, d * mt], I32, kind="ExternalInput")
    idx2 = nc.dram_tensor("idx2", [128, d * mt], I32, kind="ExternalInput")
    bt1 = bt2 = None
    if cfg.has_b1:
        bt1 = nc.dram_tensor("bt1", [1, g], BF16, kind="ExternalInput")
    if cfg.has_b2:
        bt2 = nc.dram_tensor("bt2", [1, g], BF16, kind="ExternalInput")
    out = nc.dram_tensor("out", [npc, f], F32, kind="ExternalOutput")

    # --- internal DRAM ---------------------------------------------------
    table1loc = nc.dram_tensor("table1loc", [npad, g], BF16)
    table1 = nc.dram_tensor("table1", [(4 if mt % 4 == 0 else 1) * C * (npad // (4 if mt % 4 == 0 else 1)), g], BF16,
                            addr_space="Shared")
    nq = 4 if mt % 4 == 0 else 1  # table2 AllGather chunks
    qrows = npad // nq
    table2loc = nc.dram_tensor("table2loc", [npad, g], BF16)
    table2 = nc.dram_tensor("table2", [nq * C * qrows, g], BF16,
                            addr_space="Shared")

    with tile.TileContext(nc) as tc, ExitStack() as ctx:
        const = ctx.enter_context(tc.tile_pool(name="const", bufs=1))
        ident = const.tile([128, 128], BF16, tag="ident", name="ident")
        make_identity(nc, ident[:])
        idx1_sb = const.tile([128, d * mt], I32, tag="idx1", name="idx1")
        nc.sync.dma_start(out=idx1_sb[:], in_=idx1[:, :])
        idx2_sb = const.tile([128, d * mt], I32, tag="idx2", name="idx2")
        nc.sync.dma_start(out=idx2_sb[:], in_=idx2[:, :])
        blin_sb = const.tile([128, kt], F32, tag="blin", name="blin")
        nc.sync.dma_start(out=blin_sb[:], in_=blinT[:, :])

        # stage-1 hidden-state (transposed) persists into the linear stage
        s1pool = ctx.enter_context(tc.tile_pool(name="s1", bufs=1))
        hT1 = s1pool.tile([128, kt * npad], BF16, tag="hT1", name="hT1")

        # ---- stage 1: table1 = x @ Wih1.T (+b1) -------------------------
        with tc.tile_pool(name="w1", bufs=1) as w1p:
            wih1_sb = _load_wtile(nc, w1p, wih1T, kt, g, name="wih1")
            b1_sb = None
            if bt1 is not None:
                b1_sb = w1p.tile([1, g], BF16, tag="b1", name="b1")
                nc.sync.dma_start(out=b1_sb[:], in_=bt1[:, :])
            _proj_phase(
                nc, tc, cfg,
                lambda m, k: xTloc[k * 128:(k + 1) * 128, m * 128:(m + 1) * 128],
                wih1_sb, table1loc, mt, b1_sb,
            )
        nq1 = 4 if mt % 4 == 0 else 1
        q1rows = npad // nq1
        for q in range(nq1):
            nc.gpsimd.collective_compute(
                "AllGather",
                mybir.AluOpType.bypass,
                replica_groups=[list(range(C))],
                ins=[table1loc[q * q1rows:(q + 1) * q1rows, :]],
                outs=[table1[q * C * q1rows:(q + 1) * C * q1rows, :]],
            )

        # ---- stage 1: LSTM over mailboxes -------------------------------
        with tc.tile_pool(name="whh1", bufs=1) as whh1p:
            whh1_sb = _load_wtile(nc, whh1p, whh1T, kt, g, name="whh1")
            _lstm_phase(nc, tc, cfg, table1, idx1_sb, whh1_sb, ident, hT1, None)

        # ---- stage 1 linear (transposed) + local stage-2 projection -----
        with tc.tile_pool(name="lin", bufs=1) as linp:
            hlin_sb = linp.tile([128, kt * npad], BF16, tag="hlin", name="hlin")
            with (
                tc.tile_pool(name="lin_w", bufs=1) as linwp,
                tc.tile_pool(name="lin_psum", bufs=2, space="PSUM") as linpp,
            ):
                wself_sb = _load_wtile(nc, linwp, wself, kt, f, name="ws")
                wneigh_sb = _load_wtile(nc, linwp, wneigh, kt, f, name="wn")
                xtme_sb = _load_wtile(nc, linwp, xTme, kt, npad, name="xtme")
                nch = math.ceil(npad / 512)
                for hk in range(kt):
                    for nc_i in range(nch):
                        nw = min(512, npad - nc_i * 512)
                        ps = linpp.tile([128, 512], F32, tag="lp", name="lp")
                        for k in range(kt):
                            nc.tensor.matmul(
                                ps[:, :nw],
                                lhsT=wself_sb[:, k * f + hk * 128: k * f + hk * 128 + 128],
                                rhs=xtme_sb[:, k * npad + nc_i * 512: k * npad + nc_i * 512 + nw],
                                start=(k == 0),
                                stop=False,
                            )
                        for k in range(kt):
                            nc.tensor.matmul(
                                ps[:, :nw],
                                lhsT=wneigh_sb[:, k * f + hk * 128: k * f + hk * 128 + 128],
                                rhs=hT1[:, k * npad + nc_i * 512: k * npad + nc_i * 512 + nw],
                                start=False,
                                stop=(k == kt - 1),
                            )
                        nc.scalar.activation(
                            hlin_sb[:, hk * npad + nc_i * 512: hk * npad + nc_i * 512 + nw],
                            ps[:, :nw], AF.Identity,
                            bias=blin_sb[:, hk:hk + 1],
                        )

            # stage-2 projection of the LOCAL node block (stationary from SBUF)
            with tc.tile_pool(name="w2", bufs=1) as w2p:
                wih2_sb = _load_wtile(nc, w2p, wih2T, kt, g, name="wih2")
                b2_sb = None
                if bt2 is not None:
                    b2_sb = w2p.tile([1, g], BF16, tag="b2", name="b2")
                    nc.sync.dma_start(out=b2_sb[:], in_=bt2[:, :])
                _proj_phase(
                    nc, tc, cfg,
                    lambda m, k: hlin_sb[:, k * npad + m * 128:
                                         k * npad + (m + 1) * 128],
                    wih2_sb, table2loc, mt, b2_sb,
                )

        # ---- all-gather the stage-2 table (chunked, overlaps proj tail) --
        for q in range(nq):
            nc.gpsimd.collective_compute(
                "AllGather",
                mybir.AluOpType.bypass,
                replica_groups=[list(range(C))],
                ins=[table2loc[q * qrows:(q + 1) * qrows, :]],
                outs=[table2[q * C * qrows:(q + 1) * C * qrows, :]],
            )

        # ---- stage 2: LSTM -> out ---------------------------------------
        with (
            tc.tile_pool(name="whh2", bufs=1) as whh2p,
            tc.tile_pool(name="s2", bufs=1) as s2pool,
        ):
            whh2_sb = _load_wtile(nc, whh2p, whh2T, kt, g, name="whh2")
            hT2 = s2pool.tile([128, kt * npad], BF16, tag="hT2", name="hT2")
            _lstm_phase(nc, tc, cfg, table2, idx2_sb, whh2_sb, ident, hT2, out)

    nc.compile()
    return nc


_CACHE: dict = {}


def _perm_gates(w):
    """Reorder gate blocks [i, f, g, o] -> [i, f, o, g] along axis 0."""
    i, f_, g, o = np.split(np.asarray(w), 4, 0)
    return np.concatenate([i, f_, o, g], 0)


def _prep_inputs(cfg: Cfg, x, src, Wih1, Whh1, bih1, bhh1, W_self, W_neigh, b,
                 Wih2, Whh2, bih2, bhh2):
    """Build the 8 per-core input maps (host-side slicing/transposition only)."""
    f, g, d, mt, npc, npad, C = cfg.f, cfg.g, cfg.d, cfg.mt, cfg.npc, cfg.npad, cfg.cores

    def bf(a):
        return np.ascontiguousarray(a, dtype=np.float32).astype(NP_BF16)

    shared = {
        "wih1T": bf(_perm_gates(Wih1).T), "whh1T": bf(_perm_gates(Whh1).T),
        "wih2T": bf(_perm_gates(Wih2).T), "whh2T": bf(_perm_gates(Whh2).T),
        "wself": bf(W_self), "wneigh": bf(W_neigh),
        "blinT": np.ascontiguousarray(
            np.asarray(b, np.float32).reshape(cfg.kt, 128).T),
    }
    if cfg.has_b1:
        shared["bt1"] = bf(_perm_gates(
            np.asarray(bih1) + np.asarray(bhh1))[None, :])
    if cfg.has_b2:
        shared["bt2"] = bf(_perm_gates(
            np.asarray(bih2) + np.asarray(bhh2))[None, :])

    src = np.asarray(src)
    nq = 4 if mt % 4 == 0 else 1
    qrows = npad // nq
    c_ = src // npc
    r_ = src % npc  # local row in the owner's padded block (< npc <= npad)
    src2 = (r_ // qrows) * (C * qrows) + c_ * qrows + (r_ % qrows)
    # table1 is sharded by npad-sized blocks (core c projects nodes
    # [c*npad, (c+1)*npad)) and all-gathered in the same chunked layout
    c1 = src // npad
    r1 = src % npad
    src1 = (r1 // qrows) * (C * qrows) + c1 * qrows + (r1 % qrows)

    in_maps = []
    for c in range(C):
        lo = c * npc
        xme = np.zeros((f, npad), NP_BF16)
        xme[:, :npc] = bf(x[lo:lo + npc].T)

        def pack(s):
            # [128, d*mt] with [p, t*mt+m] = s[m*128+p, t] for this core
            a = np.zeros((npad, d), np.int32)
            a[:npc] = s[lo:lo + npc]
            return np.ascontiguousarray(
                a.reshape(mt, 128, d).transpose(1, 2, 0).reshape(128, d * mt))

        xloc = np.zeros((f, npad), NP_BF16)
        pl = min(cfg.n - c * npad, npad) if c * npad < cfg.n else 0
        if pl > 0:
            xloc[:, :pl] = bf(x[c * npad:c * npad + pl].T)
        m = dict(shared)
        m["xTme"] = xme
        m["xTloc"] = xloc
        m["idx1"] = pack(src1.astype(np.int64))
        m["idx2"] = pack(src2.astype(np.int64))
        in_maps.append(m)
    return in_maps


def run(inputs: dict, trace: bool = False):
    """Build (cached), run on 8 cores, return (output [N,F] fp32, results)."""
    x = np.asarray(inputs["x"])
    n, f = x.shape
    d = np.asarray(inputs["src"]).shape[1]
    cfg = Cfg(
        n=n, d=d, f=f, cores=8,
        has_b1=bool(np.any(inputs["bih1"]) or np.any(inputs["bhh1"])),
        has_b2=bool(np.any(inputs["bih2"]) or np.any(inputs["bhh2"])),
    )
    if cfg not in _CACHE:
        _CACHE[cfg] = build(cfg)
    nc = _CACHE[cfg]
    in_maps = _prep_inputs(
        cfg, x, inputs["src"], inputs["Wih1"], inputs["Whh1"], inputs["bih1"],
        inputs["bhh1"], inputs["W_self"], inputs["W_neigh"], inputs["b"],
        inputs["Wih2"], inputs["Whh2"], inputs["bih2"], inputs["bhh2"],
    )
    res = run_bass_kernel_spmd(nc, in_maps, core_ids=list(range(cfg.cores)),
                               trace=trace)
    outp = np.concatenate([res.results[c]["out"] for c in range(cfg.cores)], 0)
    return np.ascontiguousarray(outp[:n], dtype=np.float32), res


def kernel(**inputs) -> np.ndarray:
    out, _ = run(inputs, trace=False)
    return out
